# revision 1
# baseline (speedup 1.0000x reference)
"""Banded dense-dilated KNN graph (k=9, band 90, dilation 1) on 8 Trainium2 cores.

Input  x: (4, 64, 8192, 1) float32.
Output e: (2, 4, 8192, 9) int32 = stack([nn_idx, center_idx]).

Algorithm notes
---------------
The reference L2-normalizes x over the 64-dim feature axis and takes, per row
i, the 9 smallest banded distances d(i,j) = |u_i|^2 + |u_j|^2 - 2 u_i.u_j for
j in [i-89, i].  After normalization |u_j|^2 == 1 +/- ~5e-7 uniformly, so the
within-row ordering is (to far below the fp32 matmul noise floor) the ordering
of the dot products u_i.u_j descending, and rank 0 is always j == i (self).
The device therefore computes, per 128-row block, the [128 x 217] window of
dot products via one fp32 PE matmul (stationary = block rows, moving = its
89-back-extended column window), masks everything outside j in [i-89, i-1]
(including self) by subtracting a 0/1e30 mask, and extracts the top-8 values
and indices per row with the DVE max8/max_index instructions.  Self (rank 0),
the first-8-row head fixup, and the center-index plane are reconstructed on
the host, which is exact.

Sharding: 8 cores = 4 batches x 2 row-halves of 4096 rows; no cross-core
communication.  Each core gets its own 4096 rows plus the 89 preceding
columns (zero padding for the batch-leading half).  On-chip the 4185 columns
are stacked into a [128 x 2137] layout (two 64-partition halves overlapping
by 89 columns) so the elementwise pre-pass runs at full partition width.
"""

import sys

import numpy as np

for _p in ("/opt/trn_rl_repo", "/root/.axon_site/_ro/trn_rl_repo"):
    if _p not in sys.path:
        sys.path.append(_p)

B = 4
D = 64
N = 8192
K = 9
LB = 90  # band width (j in [i-89, i])
W = LB - 1  # 89 back-columns
HALF = N // 2  # rows per core
NCOLS = W + HALF  # 4185 input columns per core
NBLK = HALF // 128  # 32 row blocks per core
WIN = 128 + W  # 217-column matmul window
HALF_BLK = NBLK // 2  # 16 blocks per stacked half
HCOLS = W + HALF_BLK * 128  # 2137 columns per stacked half
BIG = 1.0e30

_CACHED = {}


MEGA = [(0, 345), (345, 768), (1113, 1024)]  # cumulative cols unlock 1/8/16 blocks per half


def _subchunks(c0, cw):
    out = []
    o = c0
    while o < c0 + cw:
        w = min(512, c0 + cw - o)
        out.append((o, w))
        o += w
    return out


def _build_masks():
    # mask[r, c] = 0 where column c is a valid neighbor of block-row r, 1e30
    # otherwise.  Valid (non-self) neighbors of global row i = r0 + r are
    # j in [i-89, i-1]  ->  c = j - (r0 - 89) in [r, r+88].
    r = np.arange(128)[:, None]
    c = np.arange(WIN)[None, :]
    valid = (c >= r) & (c <= r + W - 1)
    m_rest = np.where(valid, 0.0, BIG).astype(np.float32)
    # Block 0 of a batch-leading half additionally requires j >= 0 (c >= 89;
    # columns 0..88 are sentinel padding).
    valid0 = valid & (c >= W)
    m_first = np.where(valid0, 0.0, BIG).astype(np.float32)
    return m_first, m_rest


def _build_bass():
    import concourse.mybir as mybir
    from concourse import bacc
    from concourse.tile import TileContext

    f32 = mybir.dt.float32
    u32 = mybir.dt.uint32
    Act = mybir.ActivationFunctionType
    Alu = mybir.AluOpType

    nc = bacc.Bacc("TRN2", target_bir_lowering=False, debug=False, num_devices=8)
    xs_d = nc.dram_tensor("xs", [D, NCOLS], f32, kind="ExternalInput")
    mf_d = nc.dram_tensor("m_first", [128, WIN], f32, kind="ExternalInput")
    mr_d = nc.dram_tensor("m_rest", [128, WIN], f32, kind="ExternalInput")
    selt_d = nc.dram_tensor("selt", [2, 128], f32, kind="ExternalInput")
    idx_d = nc.dram_tensor("idx_out", [HALF, 8], u32, kind="ExternalOutput")

    with TileContext(nc) as tc:
        with (
            tc.tile_pool(name="big", bufs=1) as big,
            tc.tile_pool(name="consts", bufs=1) as consts,
            tc.tile_pool(name="work", bufs=4) as work,
            tc.tile_pool(name="gbp", bufs=4) as gbp,
            tc.tile_pool(name="nrow", bufs=4) as nrow,
            tc.tile_pool(name="pss", bufs=2, space="PSUM") as pss,
            tc.tile_pool(name="psg", bufs=2, space="PSUM") as psg,
            tc.tile_pool(name="psd", bufs=4, space="PSUM") as psd,
            tc.tile_pool(name="sco", bufs=8) as sco,
            tc.tile_pool(name="out8", bufs=8) as out8,
        ):
            X = big.tile([128, HCOLS], f32, tag="X")
            U = big.tile([128, HCOLS], f32, tag="U")
            # Batched top-8 indices for all 32 blocks; one store at the end.
            IDX = big.tile([128, NBLK * 8], u32, tag="IDX")
            # Two stacked halves, overlapping by the 89 window columns,
            # loaded chunk-by-chunk on alternating queues so the pre-pass
            # can start immediately.
            # Warm both ACT function tables (Square/Sqrt, Copy) immediately so
            # the ~1.3us table loads overlap the input DMAs.
            warm = consts.tile([2, 2], f32, tag="warm")
            nc.vector.memset(warm[:], 1.0)
            nc.scalar.activation(warm[:], warm[:], Act.Square)
            nc.scalar.activation(warm[:], warm[:], Act.Sqrt)

            for mi, (c0, cw) in enumerate(MEGA):
                # Last megachunk rides the Pool SWDGE queue so the first
                # chunks land sooner on SP (Pool's first compute is late).
                eng = nc.sync if mi < len(MEGA) - 1 else nc.gpsimd
                eng.dma_start(X[0:64, c0 : c0 + cw], xs_d[:, c0 : c0 + cw])
                eng.dma_start(
                    X[64:128, c0 : c0 + cw],
                    xs_d[:, HALF_BLK * 128 + c0 : HALF_BLK * 128 + c0 + cw],
                )

            mf = consts.tile([128, WIN], f32, tag="mf")
            nc.gpsimd.dma_start(mf[:], mf_d[:])
            mr = consts.tile([128, WIN], f32, tag="mr")
            nc.gpsimd.dma_start(mr[:], mr_d[:])
            # Per-half column-sum selector: ones in column h for partition
            # half h, so one K=128 matmul yields both halves' sums.
            sel = consts.tile([128, 2], f32, tag="sel")
            nc.vector.memset(sel[:], 0.0)
            nc.vector.memset(sel[0:64, 0:1], 1.0)
            nc.vector.memset(sel[64:128, 1:2], 1.0)
            # Transposed selector (host-provided: sub-partition memsets are
            # not addressable): broadcasts a [2, cw] row pair to the matching
            # 64-partition halves via one K=2 matmul.
            selT = consts.tile([2, 128], f32, tag="selT")
            nc.gpsimd.dma_start(selT[:], selt_d[:])

            def pre_chunk(c0, cw, first=False):
                # One wide Square, then per-<=512 subchunks (PSUM bank
                # limit): column sums, sqrt, reciprocal, then a K=2 selector
                # matmul broadcasts the per-column scales to both partition
                # halves for the normalization multiply.
                sl = slice(c0, c0 + cw)
                xx = work.tile([128, cw], f32, tag="xx")
                if first:
                    # ACT is still loading function tables; DVE is free.
                    nc.vector.tensor_tensor(xx[:], X[:, sl], X[:, sl], op=Alu.mult)
                else:
                    nc.scalar.activation(xx[:], X[:, sl], Act.Square)
                for s0, sw in _subchunks(c0, cw):
                    ssl = slice(s0, s0 + sw)
                    xsl = slice(s0 - c0, s0 - c0 + sw)
                    ssp = pss.tile([2, sw], f32, tag="ssp")
                    nc.tensor.matmul(
                        ssp[:], lhsT=sel[:], rhs=xx[:, xsl], start=True, stop=True
                    )
                    # No max(norm, eps) clamp needed on-device: the host
                    # fills the batch-leading pad columns with a unit
                    # sentinel, so every column has norm >= ~1.
                    ns = nrow.tile([2, sw], f32, tag="ns")
                    nc.scalar.activation(ns[:], ssp[:], Act.Sqrt)
                    g2 = nrow.tile([2, sw], f32, tag="g2")
                    nc.vector.reciprocal(g2[:], ns[:])
                    gps = psg.tile([128, sw], f32, tag="gps")
                    nc.tensor.matmul(
                        gps[:], lhsT=selT[:], rhs=g2[:], start=True, stop=True
                    )
                    gb = gbp.tile([128, sw], f32, tag="gb")
                    nc.scalar.activation(gb[:], gps[:], Act.Copy)
                    nc.gpsimd.tensor_tensor(U[:, ssl], X[:, ssl], gb[:], op=Alu.mult)

            def main_block(t):
                p0 = 64 * (t // HALF_BLK)
                tl = t % HALF_BLK
                a0 = W + 128 * tl
                w0 = 128 * tl
                pd = psd.tile([128, WIN], f32, tag="pd")
                nc.tensor.matmul(
                    pd[:],
                    lhsT=U[p0 : p0 + 64, a0 : a0 + 128],
                    rhs=U[p0 : p0 + 64, w0 : w0 + WIN],
                    start=True,
                    stop=True,
                )
                dsb = sco.tile([128, WIN], f32, tag="dsb")
                nc.scalar.activation(dsb[:], pd[:], Act.Copy)
                m = mf if t == 0 else mr
                sc = sco.tile([128, WIN], f32, tag="sc")
                nc.gpsimd.tensor_tensor(sc[:], dsb[:], m[:], op=Alu.subtract)
                vals = out8.tile([128, 8], f32, tag="vals")
                nc.vector.max(out=vals[:], in_=sc[:])
                nc.vector.max_index(
                    out=IDX[:, 8 * t : 8 * (t + 1)], in_max=vals[:], in_values=sc[:]
                )

            # Wave-pipelined emission: each megachunk's normalization is
            # followed by the block pairs it unlocks; later waves overlap
            # earlier main work.
            # Batched index stores (one per 8-block group, emitted as soon
            # as a group's blocks are all done): dram row 128*t + r, col k
            # <- IDX[r, 8*t + k].
            idx_rtk = idx_d.ap().rearrange("(t r) k -> r t k", t=NBLK, r=128)

            def store_group(gi):
                nc.sync.dma_start(
                    idx_rtk[:, slice(8 * gi, 8 * (gi + 1)), :],
                    IDX[:, 64 * gi : 64 * (gi + 1)],
                )

            unlocked = [1, 8, HALF_BLK]
            emitted = 0
            for mi, (c0, cw) in enumerate(MEGA):
                pre_chunk(c0, cw, first=(mi == 0))
                while emitted < unlocked[mi]:
                    main_block(emitted)
                    main_block(HALF_BLK + emitted)
                    emitted += 1
                    if emitted == 8:
                        store_group(0)  # blocks 0-7
                        store_group(2)  # blocks 16-23
            store_group(1)  # blocks 8-15
            store_group(3)  # blocks 24-31

    nc.finalize()
    return nc


LAST_EXEC_NS = None


def kernel(x: np.ndarray) -> np.ndarray:
    global LAST_EXEC_NS
    import os

    from concourse import bass_utils

    if "nc" not in _CACHED:
        _CACHED["nc"] = _build_bass()
        _CACHED["masks"] = _build_masks()
    nc = _CACHED["nc"]
    m_first, m_rest = _CACHED["masks"]

    x = np.asarray(x)
    assert x.shape == (B, D, N, 1) and x.dtype == np.float32
    xm = x[:, :, :, 0]  # (B, D, N)

    in_maps = []
    for core in range(8):
        b, h = core // 2, core % 2
        if h == 0:
            # Unit sentinel in the pad region: keeps norms ~8 (no eps clamp
            # needed on-device); pad columns are masked out regardless.
            xs = np.concatenate(
                [np.ones((D, W), np.float32), xm[b, :, 0:HALF]], axis=1
            )
        else:
            xs = np.ascontiguousarray(xm[b, :, HALF - W : N])
        selt = np.zeros((2, 128), np.float32)
        selt[0, 0:64] = 1.0
        selt[1, 64:128] = 1.0
        in_maps.append(
            {
                "xs": xs,
                "m_first": m_first if h == 0 else m_rest,
                "m_rest": m_rest,
                "selt": selt,
            }
        )

    trace = os.environ.get("KNN_TRACE", "0") == "1"
    res = bass_utils.run_bass_kernel_spmd(nc, in_maps, core_ids=list(range(8)), trace=trace)
    LAST_EXEC_NS = res.exec_time_ns

    # --- host-side unshard + index reconstruction (exact) ---
    nn = np.empty((B, N, K), np.int64)
    rows = np.arange(HALF)
    offs = (rows // 128) * 128 - W  # window base per local row block
    for core in range(8):
        b, h = core // 2, core % 2
        start = h * HALF
        c = res.results[core]["idx_out"].astype(np.int64)  # (HALF, 8)
        nn[b, start : start + HALF, 1:] = c + (start + offs)[:, None]
    nn[:, :, 0] = np.arange(N)[None, :]
    # Head fixup: row i < 8 has only i valid non-self neighbors; reference
    # fills columns k > i with the self index.
    for i in range(K - 1):
        nn[:, i, i + 1 :] = i
    center = np.broadcast_to(np.arange(N)[None, :, None], (B, N, K))
    return np.stack([nn, center], axis=0).astype(np.int32)

